# revision 44
# baseline (speedup 1.0000x reference)
# Trainium2 Bass kernel for nn_DCLS_semi_DANNLayer (DCLS gaussian convs + BN +
# LIF scan + inhibitory linear), data-parallel over batch on 8 NeuronCores.
#
# Design notes:
# - DCLS kernels are built exactly on the host and DMA'd in; taps whose
#   gaussian weight is negligible are skipped with a per-(branch, chunk)
#   error-budgeted window computed from the actual inputs at compile time.
# - x is loaded contiguously (5 channels per partition; kernel rows permuted
#   on host to match) so DMA descriptors are 6KB, not 1.2KB.
# - The leftover 60-channel chunk packs two taps per matmul via a one-tap-
#   shifted x copy in partitions 60:120 (filled by an SBUF->SBUF DMA).
# - Matmuls cover two batches x 256 t (512-col PSUM bank); the 20-col tails
#   run once per unit over all 8 batches. The inhibitory linear for the
#   second exc slice accumulates (negated weights) directly into conv PSUM.
# - PSUM drains run on the Scalar engine; BN + the 276-step LIF scan run on
#   Vector, shadowed under the exc sweeps; BN stats are all-reduced across
#   cores while the first exc sweep runs.
#
# Self-contained: hardcodes all shapes; takes FULL inputs, returns FULL output.
import numpy as np

import concourse.bacc as bacc
import concourse.bass as bass
import concourse.mybir as mybir
import concourse.tile as tile
from concourse import bass_utils


# ---- problem constants (hardcoded per spec) ----
N_CORES = 8
B, CI, T = 64, 700, 300
D = 25
TP = T - D + 1            # 276
NE, NI = 256, 128
BL = B // N_CORES         # 8 batches per core
N_LOC = BL * TP           # 2208, (b, t) layout
TAU = 2.0
A_DECAY = 1.0 - 1.0 / TAU  # 0.5
VTH = 1.0
BN_EPS = 1e-5
LIM = D // 2              # 12
TS = 256                  # per-batch columns in the paired matmul
TR = TP - TS              # 20 tail columns

N_CHUNK = 6               # ch0..ch4 (5-packed channels 0:640) + tail (640:700)
ROWS = [128, 128, 128, 128, 128, 120]
CH_ORDER = (5, 0, 1, 2, 3, 4)

BUDGET_EXC = 0.08         # abs std of dropped-tap noise (output absmax ~100)
BUDGET_INH = 0.002        # fp16 quantization adds ~0.0067 on top of tap drop

F32 = mybir.dt.float32
F32R = mybir.dt.float32r
F16 = mybir.dt.float16
ALU = mybir.AluOpType
ACTF = mybir.ActivationFunctionType

_CACHE: dict = {}


# ---------------------------------------------------------------- host side
def _build_dcls_host(W, P, SIG):
    """Exact DCLS 'gauss' kernel, matching the reference math. (O,I,1)->(O,I,D)"""
    j = np.arange(D, dtype=np.float32)
    Pc = np.clip(P[:, :, 0], -LIM, LIM).astype(np.float32) + np.float32(LIM)
    sig = np.abs(SIG[:, :, 0]).astype(np.float32) + np.float32(0.27)
    g = np.exp(np.float32(-0.5) * ((j[None, None, :] - Pc[..., None]) / sig[..., None]) ** 2)
    g = g / (g.sum(-1, keepdims=True) + np.float32(1e-7))
    return np.abs(W[:, :, 0]).astype(np.float32)[..., None] * g


def _chunk_idx():
    perm = 5 * np.arange(128)
    return [perm + ch for ch in range(5)] + [np.arange(640, 700)]


def _chunk_windows(k, budget):
    """Per-chunk contiguous tap windows (d0, L) such that for every output
    channel the total dropped-tap noise std (x ~ N(0,1)) is within budget."""
    var = np.stack([(k[:, ix, :].astype(np.float64) ** 2).sum(1)
                    for ix in _chunk_idx()], 1)          # (O, 6, D)
    tot = var.sum(1)                                     # (O, D)
    total = tot.sum(1)                                   # (O,)
    B2 = budget * budget
    # minimal global window first
    g0, gL = 0, D
    for L in range(2, D + 1):
        done = False
        for d0 in range(0, D - L + 1):
            if (total - tot[:, d0:d0 + L].sum(1)).max() <= B2:
                g0, gL = d0, L
                done = True
                break
        if done:
            break
    win = [[g0, g0 + gL - 1] for _ in range(N_CHUNK)]
    V = total - tot[:, g0:g0 + gL].sum(1)
    # greedy per-chunk edge shrink
    while True:
        best = None
        for c in range(N_CHUNK):
            a, b_ = win[c]
            if b_ - a + 1 <= 1:
                continue
            for side, d in ((0, a), (1, b_)):
                vmax = (V + var[:, c, d]).max()
                if vmax <= B2 and (best is None or vmax < best[0]):
                    best = (vmax, c, side, d)
        if best is None:
            break
        _, c, side, d = best
        V = V + var[:, c, d]
        if side == 0:
            win[c][0] += 1
        else:
            win[c][1] -= 1
    return tuple((a, b_ - a + 1) for a, b_ in win)


def _sweep_layout(wins):
    """Column offsets per chunk within a sweep's kt block: [tail|ch0..ch4]."""
    np5 = (wins[5][1] + 1) // 2
    offs = [0] * N_CHUNK
    offs[5] = 0
    off = np5 * 128
    for c in range(5):
        offs[c] = off
        off += wins[c][1] * 128
    return offs, off


def _pack_segments(kall, sched):
    idxs = _chunk_idx()
    blocks = []
    for (o0, wins) in sched:
        offs, width = _sweep_layout(wins)
        blk = np.zeros((128, width), dtype=np.float32)
        d5, L5 = wins[5]
        taps5 = list(range(d5, d5 + L5))
        np5 = (L5 + 1) // 2
        buf = np.zeros((128, np5, 128), dtype=np.float32)
        buf[0:60] = np.transpose(kall[o0:o0 + 128][:, idxs[5], :]
                                 [:, :, taps5[0::2]], (1, 2, 0))
        if taps5[1::2]:
            buf[60:120, :len(taps5[1::2])] = np.transpose(
                kall[o0:o0 + 128][:, idxs[5], :][:, :, taps5[1::2]], (1, 2, 0))
        blk[:, 0:np5 * 128] = buf.reshape(128, np5 * 128)
        for c in range(5):
            d0, L = wins[c]
            sub = np.transpose(kall[o0:o0 + 128][:, idxs[c], d0:d0 + L],
                               (1, 2, 0))
            blk[:, offs[c]:offs[c] + L * 128] = sub.reshape(128, L * 128)
        blocks.append(blk)
    return np.concatenate(blocks, axis=1)


# ---------------------------------------------------------------- device side
def _build_nc(sched):
    nc = bacc.Bacc("TRN2", target_bir_lowering=False, debug=False,
                   num_devices=N_CORES)
    # stats exchange semaphores (allocated outside the tile context so the
    # post-context clear can reference them)
    sem_arrive = nc.alloc_semaphore("stats_arrive")
    sem_sent = nc.alloc_semaphore("stats_sent")
    sem_bar = nc.alloc_semaphore("end_align")

    layouts = [_sweep_layout(wins) for (_, wins) in sched]
    widths = [w for (_, w) in layouts]
    bases = [sum(widths[:s]) for s in range(3)]

    xs_d = nc.dram_tensor("xs", [BL, CI, T], F16, kind="ExternalInput")
    kt_d = nc.dram_tensor("kt", [128, sum(widths)], F16, kind="ExternalInput")
    wei_d = nc.dram_tensor("wei", [NI, NE], F16, kind="ExternalInput")
    bng_d = nc.dram_tensor("bng", [NI, 1], F32, kind="ExternalInput")
    bnb_d = nc.dram_tensor("bnb", [NI, 1], F32, kind="ExternalInput")
    # o-major output layout: per-partition rows are BL*TP*4B contiguous, so
    # stores need only 128 large descriptors; host transposes back to (B,O,T)
    out_d = nc.dram_tensor("out", [NE, BL, TP], F32, kind="ExternalOutput")

    def taps_of(s, c):
        d0, L = sched[s][1][c]
        return list(range(d0, d0 + L))

    with tile.TileContext(nc) as tc:
        import contextlib

        with contextlib.ExitStack() as ctx:
            singles = ctx.enter_context(tc.tile_pool(name="singles", bufs=1))
            ktpool = ctx.enter_context(tc.tile_pool(name="ktpool", bufs=10))
            ppool = ctx.enter_context(
                tc.tile_pool(name="ppool", bufs=8, space="PSUM"))

            # ---- persistent SBUF tiles ----
            xtm = singles.tile([128, BL, 5, T], F16)   # channels 0:640, 5/part
            xt5 = singles.tile([128, BL, T], F16)      # channels 640:700 + shift
            inh = singles.tile([NI, N_LOC], F32)        # (b, t) layout
            inh3 = inh.rearrange("p (b t) -> p b t", t=TP)
            spk = singles.tile([NI, N_LOC], F16)
            exc0 = singles.tile([128, BL, TP], F32)
            exc1 = singles.tile([128, BL, TP], F32)
            wei_neg = singles.tile([NI, NE], F16)      # -|w_exc_inh|.T (host)
            bng = singles.tile([NI, 1], F32)
            bnb = singles.tile([NI, 1], F32)
            stats = singles.tile([NI, 2], F32)
            gst = singles.tile([NI, 2], F32)
            smalls = singles.tile([NI, 8], F32)
            w_st = singles.tile([NI, BL], F32)
            # per-core stats land here: slot k (cols 2k:2k+2) holds the stats
            # of peer (self XOR k); slot 0 is written locally
            recv = singles.tile([NI, 2 * N_CORES], F32)

            # per-(sweep, chunk) kt tiles: tile-granular DMA dependencies so a
            # matmul only waits for its own chunk's columns
            kt_tiles = {}

            ktw = max((((L_ + 1) // 2) if c_ == 5 else L_) * 128
                      for (_, ws) in sched for c_, (_, L_) in enumerate(ws))

            def load_seg(s, c):
                L = sched[s][1][c][1]
                ncols = (((L + 1) // 2) if c == 5 else L) * 128
                a = layouts[s][0][c]
                t_ = ktpool.tile([128, ktw], F16, tag="kt", name=f"kt{s}{c}")
                nc.sync.dma_start(
                    out=t_[:, :ncols],
                    in_=kt_d.ap()[:, bases[s] + a:bases[s] + a + ncols])
                kt_tiles[(s, c)] = t_

            # ---- head DMAs (sync engine; order = priority) ----
            # sweep-0 pass 0 needs: xt5 batch 0-3, the chunk-5 kernel
            # columns, xtm batch 0, then the remaining sweep-0 chunks.
            def load_x(b_):
                nc.sync.dma_start(out=xtm[:, b_], in_=xs_d.ap()[b_, 0:640]
                                  .rearrange("(p c) t -> p c t", c=5))

            nc.sync.dma_start(
                out=xt5[0:60, 0:4], in_=xs_d.ap()[0:4, 640:700]
                .rearrange("b i t -> i b t"))
            nc.sync.dma_start(out=xt5[60:120, 0:4, 0:T - 1],
                              in_=xt5[0:60, 0:4, 1:T])
            load_seg(0, 5)
            load_x(0)
            for c_ in (0, 1, 2, 3, 4):
                load_seg(0, c_)
            load_x(1)
            load_x(2)
            load_x(3)
            nc.sync.dma_start(
                out=xt5[0:60, 4:8], in_=xs_d.ap()[4:8, 640:700]
                .rearrange("b i t -> i b t"))
            nc.sync.dma_start(out=xt5[60:120, 4:8, 0:T - 1],
                              in_=xt5[0:60, 4:8, 1:T])
            for b_ in range(4, BL):
                load_x(b_)
            nc.sync.dma_start(out=wei_neg[:], in_=wei_d.ap())
            nc.sync.dma_start(out=bng[:], in_=bng_d.ap())
            nc.sync.dma_start(out=bnb[:], in_=bnb_d.ap())
            for s_ in (1, 2):
                for c_ in CH_ORDER:
                    load_seg(s_, c_)

            nc.vector.memset(w_st[:], 0.0)
            eps_c = smalls[:, 7:8]
            nc.vector.memset(eps_c, BN_EPS)

            def rhs(c, b_, t0, t1):
                if c < 5:
                    return xtm[:, b_:b_ + 1, c, t0:t1]
                return xt5[:120, b_:b_ + 1, t0:t1]

            # ---- sweep emitter ----
            # One 276-col pass per batch over ALL units: every matmul is a
            # full-rate >=256-col fp32r op into a single PSUM bank, there are
            # no narrow tails, and each pass drains right after it finishes
            # so the 6-buf pool pipelines across passes and sweeps.
            def units_of(s):
                out = []
                for c in CH_ORDER:
                    taps = taps_of(s, c)
                    n_units = len(taps) if c < 5 else (len(taps) + 1) // 2
                    for j in range(n_units):
                        d = taps[j] if c < 5 else taps[2 * j]
                        out.append((c, j, d))
                return out

            # Drains: sweep 0 uses the Scalar engine (nothing remote-gated
            # precedes it in the ACT stream); sweeps 1-2 drain via DMA so no
            # conv-side PSUM recycling ever waits on a Scalar tick that the
            # remote-stats-gated Sqrt could block.
            def emit_sweep(s, dst3, dma_drain=False):
                units = units_of(s)
                nu = len(units)
                for b_ in range(BL):
                    pt = ppool.tile([128, TP], F32, tag="pp",
                                    name=f"p{s}{b_}")
                    for i, (c, j, d) in enumerate(units):
                        lhsT = kt_tiles[(s, c)][:ROWS[c],
                                                j * 128:(j + 1) * 128]
                        nc.tensor.matmul(
                            pt[:], lhsT, rhs(c, b_, d, d + TP),
                            start=(i == 0), stop=(i == nu - 1))
                    nc.scalar.copy(out=dst3[:, b_, :], in_=pt[:])

            # stats all-gather via direct remote DMA (XOR-relative slots):
            # core c sends its [NI,2] stats into slot k of peer c^k. ~3us
            # vs ~40us for the firmware mesh AllReduce on a 1KB buffer.
            # Descriptor generation (7 x ~830ns of Pool work) happens HERE,
            # at kernel start, on SWDGE queue 1; only the trigger waits for
            # the stats. The entry-barrier / arrival waits are attached
            # post-scheduling (below) — the tile scheduler's single-core sim
            # cannot satisfy remotely-incremented semaphores.
            for k in range(1, N_CORES):
                rd = [None] * 8
                rd[k] = (0, k)
                nc.gpsimd.remote_dma_broadcast(
                    out_ap=recv[:, 2 * k:2 * k + 2], in_ap=stats[:, 0:2],
                    remote_sem=sem_arrive, local_sem=sem_sent, rdests=rd)

            # ---------- sweep 0: inhibitory ----------
            emit_sweep(0, inh3)
            nc.vector.reduce_sum(stats[:, 0:1], inh[:],
                                 axis=mybir.AxisListType.X)
            nc.vector.scalar_tensor_tensor(
                spk[:], inh[:], 0.0, inh[:], ALU.bypass, ALU.mult,
                accum_out=stats[:, 1:2])
            # Order the trigger after the stats write: a Pool-engine read of
            # stats into a dummy tile (RAW on stats), then a WAW between that
            # dummy and the trigger's signals_writable.
            dummy = singles.tile([NI, 2], F32)
            nc.gpsimd.tensor_copy(out=dummy[:], in_=stats[:])
            stats_trigger = nc.gpsimd.trigger_dma(
                count=None, signals_writable=[dummy[:]])
            nc.vector.tensor_copy(out=recv[:, 0:2], in_=stats[:, 0:2])

            # ---------- sweeps 1+2: excitatory ----------
            emit_sweep(1, exc0, dma_drain=True)
            emit_sweep(2, exc1, dma_drain=True)

            # BN math after the stats arrive from all peers (wait rides on
            # the first reduce op). Everything downstream of the remote wait
            # is emitted AFTER both exc sweeps so the scheduler cannot place
            # remote-gated ops mid-stream, where their in-order engines would
            # block tick-based waits of unrelated conv work.
            sg = smalls[:, 4:5]
            b2 = smalls[:, 6:7]
            stats_reduce = nc.vector.tensor_add(
                recv[:, 0:8], recv[:, 0:8], recv[:, 8:16])
            nc.vector.tensor_add(recv[:, 0:4], recv[:, 0:4], recv[:, 4:8])
            nc.vector.tensor_add(recv[:, 0:2], recv[:, 0:2], recv[:, 2:4])
            ninv = 1.0 / (N_LOC * N_CORES)
            nc.vector.tensor_scalar_mul(gst[:], recv[:, 0:2], ninv)
            gmean = gst[:, 0:1]
            gex2 = gst[:, 1:2]
            msq = smalls[:, 0:1]
            nc.vector.tensor_mul(msq, gmean, gmean)
            var = smalls[:, 1:2]
            nc.vector.tensor_sub(var, gex2, msq)
            stdv = smalls[:, 2:3]
            nc.scalar.activation(stdv, var, ACTF.Sqrt, bias=eps_c)
            rstd = smalls[:, 3:4]
            nc.vector.reciprocal(rstd, stdv)
            nc.vector.tensor_mul(sg, rstd, bng[:])
            ms = smalls[:, 5:6]
            nc.vector.tensor_mul(ms, gmean, sg)
            nc.vector.tensor_sub(b2, bnb[:], ms)

            # ---------- BN apply + LIF scan (Vector, overlaps sweep 2) ----
            nc.vector.scalar_tensor_tensor(
                inh[:], inh[:], sg, b2.broadcast_to([NI, N_LOC]),
                ALU.mult, ALU.add)
            for t_i in range(TP):
                vsl = inh3[:, :, t_i]
                nc.vector.scalar_tensor_tensor(
                    vsl, w_st[:], A_DECAY, vsl, ALU.mult, ALU.add)
                nc.vector.scalar_tensor_tensor(
                    w_st[:], vsl, VTH, vsl, ALU.is_lt, ALU.mult)
            nc.vector.tensor_single_scalar(spk[:], inh[:], VTH, ALU.is_ge)

            # ---------- inhibitory linear for both exc halves ----------
            # All lin matmuls sit after every conv in the in-order PE queue,
            # so a wait for spikes cannot stall conv work. Lin tiles reuse
            # the (drained) conv PSUM pool; adds run on Vector (GpSimd
            # cannot read PSUM); stores fire per half (exc0 on the ACT
            # ring, exc1 on the sync ring).
            for half, (dstE, o0) in enumerate(((exc0, 0), (exc1, 128))):
                lw = wei_neg[:, o0:o0 + 128]
                for b_ in range(BL):
                    lp = ppool.tile([128, TP], F32, tag="pp",
                                    name=f"l{half}{b_}")
                    nc.tensor.matmul(lp[:], lw, spk[:, b_ * TP:(b_ + 1) * TP],
                                     start=True, stop=True)
                    nc.vector.tensor_add(dstE[:, b_, :], dstE[:, b_, :],
                                         lp[:])
                    if b_ % 4 == 3:
                        if half == 0:
                            nc.scalar.dma_start(
                                out=out_d.ap()[0:128, b_ - 3:b_ + 1, :],
                                in_=exc0[:, b_ - 3:b_ + 1, :])
                        else:
                            nc.sync.dma_start(
                                out=out_d.ap()[128:256, b_ - 3:b_ + 1, :],
                                in_=exc1[:, b_ - 3:b_ + 1, :])

            # End-of-run alignment: every core bumps every core's end_align
            # sem. The post-epilogue wait below parks early cores until the
            # slowest finishes, so the NEXT launch starts aligned and no
            # core pays a big wait-for-peer-stats bubble mid-kernel.
            dummy2 = singles.tile([NI, 2], F32)
            nc.gpsimd.tensor_copy(out=dummy2[:], in_=exc1[:, BL - 1, 0:2])
            nc.gpsimd.remote_sem_update_broadcast(
                sem_bar, sem_sent, rdests=[(0, k) for k in range(N_CORES)])
            nc.gpsimd.trigger_dma(count=None, signals_writable=[dummy2[:]])

    # Post-scheduling: attach the HW-only semaphore waits the scheduler's
    # sim can't satisfy. The trigger waits for the kernel-entry barrier
    # (all peers started this run, so their preamble state is clean); the
    # first reduce op waits for all 7 peers' stats to have landed.
    stats_trigger.wait_op(nc._bir_kernel_barrier_sem,
                          nc.bir_kernel_barrier_sem_inc, "sem-ge", check=False)
    nc._bir_kernel_barrier_sem_replica_groups.append(set(range(N_CORES)))
    stats_reduce.wait_op(sem_arrive, 2 * (N_CORES - 1), "sem-ge", check=False)
    # Post-epilogue (after the all-engine barrier): wait for all peers'
    # end-align arms, then reset the exchange semaphores for the next run.
    nc.gpsimd.wait_ge(sem_bar, 2 * N_CORES)
    nc.clear_and_free_semaphores([sem_arrive, sem_sent, sem_bar])
    nc.compile()
    return nc


def kernel(x, W_inh, P_inh, SIG_inh, W_exc, P_exc, SIG_exc, w_exc_inh,
           bn_gamma, bn_beta):
    ke = _build_dcls_host(np.asarray(W_exc), np.asarray(P_exc),
                          np.asarray(SIG_exc))        # (256, 700, D)
    ki = _build_dcls_host(np.asarray(W_inh), np.asarray(P_inh),
                          np.asarray(SIG_inh))        # (128, 700, D)
    wins_e = _chunk_windows(ke, BUDGET_EXC)
    wins_i = _chunk_windows(ki, BUDGET_INH)
    kall = np.concatenate([ke, ki], axis=0)
    # sweeps: (o_offset into kall, per-chunk windows) in order inh, exc0, exc1
    sched = ((256, wins_i), (0, wins_e), (128, wins_e))

    if _CACHE.get("key") != sched:
        _CACHE["nc"] = _build_nc(sched)
        _CACHE["key"] = sched
    nc = _CACHE["nc"]

    kt = _pack_segments(kall, sched).astype(np.float16)
    x = np.ascontiguousarray(
        np.asarray(x, dtype=np.float32).astype(np.float16))
    wei = np.ascontiguousarray(
        -np.abs(np.asarray(w_exc_inh, dtype=np.float32)).T
        .astype(np.float16))
    bng = np.asarray(bn_gamma, dtype=np.float32).reshape(NI, 1)
    bnb = np.asarray(bn_beta, dtype=np.float32).reshape(NI, 1)

    shared = {"kt": kt, "wei": wei, "bng": bng, "bnb": bnb}
    in_maps = []
    for c in range(N_CORES):
        m = dict(shared)
        m["xs"] = np.ascontiguousarray(x[c * BL:(c + 1) * BL])
        in_maps.append(m)

    _CACHE["in_maps"] = in_maps
    res = bass_utils.run_bass_kernel_spmd(nc, in_maps,
                                          core_ids=list(range(N_CORES)))
    # device emits (NE, BL, TP); transpose back to (BL, NE, TP) per core
    out = np.concatenate(
        [np.transpose(res.results[c]["out"], (1, 0, 2))
         for c in range(N_CORES)], axis=0)
    return np.ascontiguousarray(out, dtype=np.float32)



# revision 47
# speedup vs baseline: 1.1135x; 1.1135x over previous
# Trainium2 Bass kernel for nn_DCLS_semi_DANNLayer (DCLS gaussian convs + BN +
# LIF scan + inhibitory linear), data-parallel over batch on 8 NeuronCores.
#
# Design notes:
# - DCLS kernels are built exactly on the host and DMA'd in; taps whose
#   gaussian weight is negligible are skipped with a per-(branch, chunk)
#   error-budgeted window computed from the actual inputs at compile time.
# - x is loaded contiguously (5 channels per partition; kernel rows permuted
#   on host to match) so DMA descriptors are 6KB, not 1.2KB.
# - The leftover 60-channel chunk packs two taps per matmul via a one-tap-
#   shifted x copy in partitions 60:120 (filled by an SBUF->SBUF DMA).
# - Matmuls cover two batches x 256 t (512-col PSUM bank); the 20-col tails
#   run once per unit over all 8 batches. The inhibitory linear for the
#   second exc slice accumulates (negated weights) directly into conv PSUM.
# - PSUM drains run on the Scalar engine; BN + the 276-step LIF scan run on
#   Vector, shadowed under the exc sweeps; BN stats are all-reduced across
#   cores while the first exc sweep runs.
#
# Self-contained: hardcodes all shapes; takes FULL inputs, returns FULL output.
import numpy as np

import concourse.bacc as bacc
import concourse.bass as bass
import concourse.mybir as mybir
import concourse.tile as tile
from concourse import bass_utils


# ---- problem constants (hardcoded per spec) ----
N_CORES = 8
B, CI, T = 64, 700, 300
D = 25
TP = T - D + 1            # 276
NE, NI = 256, 128
BL = B // N_CORES         # 8 batches per core
N_LOC = BL * TP           # 2208, (b, t) layout
TAU = 2.0
A_DECAY = 1.0 - 1.0 / TAU  # 0.5
VTH = 1.0
BN_EPS = 1e-5
LIM = D // 2              # 12
TS = 256                  # per-batch columns in the paired matmul
TR = TP - TS              # 20 tail columns

N_CHUNK = 6               # ch0..ch4 (5-packed channels 0:640) + tail (640:700)
ROWS = [128, 128, 128, 128, 128, 120]
CH_ORDER = (5, 0, 1, 2, 3, 4)

BUDGET_EXC = 0.08         # abs std of dropped-tap noise (output absmax ~100)
BUDGET_INH = 0.002        # fp16 quantization adds ~0.0067 on top of tap drop

F32 = mybir.dt.float32
F32R = mybir.dt.float32r
F16 = mybir.dt.float16
ALU = mybir.AluOpType
ACTF = mybir.ActivationFunctionType

_CACHE: dict = {}


# ---------------------------------------------------------------- host side
def _build_dcls_host(W, P, SIG):
    """Exact DCLS 'gauss' kernel, matching the reference math. (O,I,1)->(O,I,D)"""
    j = np.arange(D, dtype=np.float32)
    Pc = np.clip(P[:, :, 0], -LIM, LIM).astype(np.float32) + np.float32(LIM)
    sig = np.abs(SIG[:, :, 0]).astype(np.float32) + np.float32(0.27)
    g = np.exp(np.float32(-0.5) * ((j[None, None, :] - Pc[..., None]) / sig[..., None]) ** 2)
    g = g / (g.sum(-1, keepdims=True) + np.float32(1e-7))
    return np.abs(W[:, :, 0]).astype(np.float32)[..., None] * g


def _chunk_idx():
    perm = 5 * np.arange(128)
    return [perm + ch for ch in range(5)] + [np.arange(640, 700)]


def _chunk_windows(k, budget):
    """Per-chunk contiguous tap windows (d0, L) such that for every output
    channel the total dropped-tap noise std (x ~ N(0,1)) is within budget."""
    var = np.stack([(k[:, ix, :].astype(np.float64) ** 2).sum(1)
                    for ix in _chunk_idx()], 1)          # (O, 6, D)
    tot = var.sum(1)                                     # (O, D)
    total = tot.sum(1)                                   # (O,)
    B2 = budget * budget
    # minimal global window first
    g0, gL = 0, D
    for L in range(2, D + 1):
        done = False
        for d0 in range(0, D - L + 1):
            if (total - tot[:, d0:d0 + L].sum(1)).max() <= B2:
                g0, gL = d0, L
                done = True
                break
        if done:
            break
    win = [[g0, g0 + gL - 1] for _ in range(N_CHUNK)]
    V = total - tot[:, g0:g0 + gL].sum(1)
    # greedy per-chunk edge shrink
    while True:
        best = None
        for c in range(N_CHUNK):
            a, b_ = win[c]
            if b_ - a + 1 <= 1:
                continue
            for side, d in ((0, a), (1, b_)):
                vmax = (V + var[:, c, d]).max()
                if vmax <= B2 and (best is None or vmax < best[0]):
                    best = (vmax, c, side, d)
        if best is None:
            break
        _, c, side, d = best
        V = V + var[:, c, d]
        if side == 0:
            win[c][0] += 1
        else:
            win[c][1] -= 1
    return tuple((a, b_ - a + 1) for a, b_ in win)


def _sweep_layout(wins):
    """Column offsets per chunk within a sweep's kt block: [tail|ch0..ch4]."""
    np5 = (wins[5][1] + 1) // 2
    offs = [0] * N_CHUNK
    offs[5] = 0
    off = np5 * 128
    for c in range(5):
        offs[c] = off
        off += wins[c][1] * 128
    return offs, off


def _pack_segments(kall, sched):
    idxs = _chunk_idx()
    blocks = []
    for (o0, wins) in sched:
        offs, width = _sweep_layout(wins)
        blk = np.zeros((128, width), dtype=np.float32)
        d5, L5 = wins[5]
        taps5 = list(range(d5, d5 + L5))
        np5 = (L5 + 1) // 2
        buf = np.zeros((128, np5, 128), dtype=np.float32)
        buf[0:60] = np.transpose(kall[o0:o0 + 128][:, idxs[5], :]
                                 [:, :, taps5[0::2]], (1, 2, 0))
        if taps5[1::2]:
            buf[60:120, :len(taps5[1::2])] = np.transpose(
                kall[o0:o0 + 128][:, idxs[5], :][:, :, taps5[1::2]], (1, 2, 0))
        blk[:, 0:np5 * 128] = buf.reshape(128, np5 * 128)
        for c in range(5):
            d0, L = wins[c]
            sub = np.transpose(kall[o0:o0 + 128][:, idxs[c], d0:d0 + L],
                               (1, 2, 0))
            blk[:, offs[c]:offs[c] + L * 128] = sub.reshape(128, L * 128)
        blocks.append(blk)
    return np.concatenate(blocks, axis=1)


# ---------------------------------------------------------------- device side
def _build_nc(sched):
    nc = bacc.Bacc("TRN2", target_bir_lowering=False, debug=False,
                   num_devices=N_CORES)
    # stats exchange semaphores (allocated outside the tile context so the
    # post-context clear can reference them)
    sem_arrive = nc.alloc_semaphore("stats_arrive")
    sem_sent = nc.alloc_semaphore("stats_sent")

    layouts = [_sweep_layout(wins) for (_, wins) in sched]
    widths = [w for (_, w) in layouts]
    bases = [sum(widths[:s]) for s in range(3)]

    xs_d = nc.dram_tensor("xs", [BL, CI, T], F16, kind="ExternalInput")
    kt_d = nc.dram_tensor("kt", [128, sum(widths)], F16, kind="ExternalInput")
    wei_d = nc.dram_tensor("wei", [NI, NE], F16, kind="ExternalInput")
    bng_d = nc.dram_tensor("bng", [NI, 1], F32, kind="ExternalInput")
    bnb_d = nc.dram_tensor("bnb", [NI, 1], F32, kind="ExternalInput")
    # o-major output layout: per-partition rows are BL*TP*4B contiguous, so
    # stores need only 128 large descriptors; host transposes back to (B,O,T)
    out_d = nc.dram_tensor("out", [NE, BL, TP], F32, kind="ExternalOutput")

    def taps_of(s, c):
        d0, L = sched[s][1][c]
        return list(range(d0, d0 + L))

    with tile.TileContext(nc) as tc:
        import contextlib

        with contextlib.ExitStack() as ctx:
            singles = ctx.enter_context(tc.tile_pool(name="singles", bufs=1))
            ktpool = ctx.enter_context(tc.tile_pool(name="ktpool", bufs=10))
            ppool = ctx.enter_context(
                tc.tile_pool(name="ppool", bufs=4, space="PSUM"))
            tpool = ctx.enter_context(
                tc.tile_pool(name="tpool", bufs=2, space="PSUM"))
            lpool = ctx.enter_context(
                tc.tile_pool(name="lpool", bufs=2, space="PSUM"))

            # ---- persistent SBUF tiles ----
            xtm = singles.tile([128, BL, 5, T], F16)   # channels 0:640, 5/part
            xt5 = singles.tile([128, BL, T], F16)      # channels 640:700 + shift
            inh = singles.tile([NI, N_LOC], F32)        # (b, t) layout
            inh3 = inh.rearrange("p (b t) -> p b t", t=TP)
            spk = singles.tile([NI, N_LOC], F16)
            exc0 = singles.tile([128, BL, TP], F32)
            exc1 = singles.tile([128, BL, TP], F32)
            wei_neg = singles.tile([NI, NE], F16)      # -|w_exc_inh|.T (host)
            bng = singles.tile([NI, 1], F32)
            bnb = singles.tile([NI, 1], F32)
            stats = singles.tile([NI, 2], F32)
            gst = singles.tile([NI, 2], F32)
            smalls = singles.tile([NI, 8], F32)
            w_st = singles.tile([NI, BL], F32)
            # per-core stats land here: slot k (cols 2k:2k+2) holds the stats
            # of peer (self XOR k); slot 0 is written locally
            recv = singles.tile([NI, 2 * N_CORES], F32)

            # per-(sweep, chunk) kt tiles: tile-granular DMA dependencies so a
            # matmul only waits for its own chunk's columns
            kt_tiles = {}

            ktw = max((((L_ + 1) // 2) if c_ == 5 else L_) * 128
                      for (_, ws) in sched for c_, (_, L_) in enumerate(ws))

            def load_seg(s, c):
                L = sched[s][1][c][1]
                ncols = (((L + 1) // 2) if c == 5 else L) * 128
                a = layouts[s][0][c]
                t_ = ktpool.tile([128, ktw], F16, tag="kt", name=f"kt{s}{c}")
                nc.sync.dma_start(
                    out=t_[:, :ncols],
                    in_=kt_d.ap()[:, bases[s] + a:bases[s] + a + ncols])
                kt_tiles[(s, c)] = t_

            # ---- head DMAs (sync engine; order = priority) ----
            # sweep-0 pass 0 needs: xt5 batch 0-3, the chunk-5 kernel
            # columns, xtm batch 0, then the remaining sweep-0 chunks.
            def load_x(b_):
                nc.sync.dma_start(out=xtm[:, b_], in_=xs_d.ap()[b_, 0:640]
                                  .rearrange("(p c) t -> p c t", c=5))

            nc.sync.dma_start(
                out=xt5[0:60, 0:4], in_=xs_d.ap()[0:4, 640:700]
                .rearrange("b i t -> i b t"))
            nc.sync.dma_start(out=xt5[60:120, 0:4, 0:T - 1],
                              in_=xt5[0:60, 0:4, 1:T])
            load_seg(0, 5)
            load_x(0)
            for c_ in (0, 1, 2, 3, 4):
                load_seg(0, c_)
            load_x(1)
            load_x(2)
            load_x(3)
            nc.sync.dma_start(
                out=xt5[0:60, 4:8], in_=xs_d.ap()[4:8, 640:700]
                .rearrange("b i t -> i b t"))
            nc.sync.dma_start(out=xt5[60:120, 4:8, 0:T - 1],
                              in_=xt5[0:60, 4:8, 1:T])
            for b_ in range(4, BL):
                load_x(b_)
            nc.sync.dma_start(out=wei_neg[:], in_=wei_d.ap())
            nc.sync.dma_start(out=bng[:], in_=bng_d.ap())
            nc.sync.dma_start(out=bnb[:], in_=bnb_d.ap())
            for s_ in (1, 2):
                for c_ in CH_ORDER:
                    load_seg(s_, c_)

            nc.vector.memset(w_st[:], 0.0)
            eps_c = smalls[:, 7:8]
            nc.vector.memset(eps_c, BN_EPS)

            def rhs(c, b0, nb, t0, t1):
                if c < 5:
                    return xtm[:, b0:b0 + nb, c, t0:t1]
                return xt5[:120, b0:b0 + nb, t0:t1]

            # ---- sweep emitter ----
            # Two-batch 256-col pair matmuls (4 PSUM tiles, tile-major so
            # consecutive matmuls share a bank) plus one 8-batch 20-col tail
            # matmul per unit: 5 instructions per unit instead of 8, which
            # keeps the engines' instruction streams inside the prefetched
            # code pages (the 8-pass form stalled ~10us on page fetches).
            def units_of(s):
                out = []
                for c in CH_ORDER:
                    taps = taps_of(s, c)
                    n_units = len(taps) if c < 5 else (len(taps) + 1) // 2
                    for j in range(n_units):
                        d = taps[j] if c < 5 else taps[2 * j]
                        out.append((c, j, d))
                return out

            def emit_sweep(s, dst3, dma_drain=False):
                units = units_of(s)
                nu = len(units)
                for q in range(2):
                    bA = 4 * q
                    pA = ppool.tile([128, 2, TS], F32, tag="pp",
                                    name=f"pA{s}{q}")
                    pB = ppool.tile([128, 2, TS], F32, tag="pp",
                                    name=f"pB{s}{q}")
                    for pt, b0 in ((pA, bA), (pB, bA + 2)):
                        for i, (c, j, d) in enumerate(units):
                            lhsT = kt_tiles[(s, c)][:ROWS[c],
                                                    j * 128:(j + 1) * 128]
                            nc.tensor.matmul(
                                pt[:], lhsT, rhs(c, b0, 2, d, d + TS),
                                start=(i == 0), stop=(i == nu - 1))
                        nc.scalar.copy(out=dst3[:, b0:b0 + 2, 0:TS],
                                       in_=pt[:])
                tt = tpool.tile([128, BL, TR], F32, tag="tp", name=f"tt{s}")
                for i, (c, j, d) in enumerate(units):
                    lhsT = kt_tiles[(s, c)][:ROWS[c], j * 128:(j + 1) * 128]
                    nc.tensor.matmul(
                        tt[:], lhsT, rhs(c, 0, BL, d + TS, d + TP),
                        start=(i == 0), stop=(i == nu - 1))
                nc.scalar.copy(out=dst3[:, :, TS:TP], in_=tt[:])

            # stats all-gather via direct remote DMA (XOR-relative slots):
            # core c sends its [NI,2] stats into slot k of peer c^k. ~3us
            # vs ~40us for the firmware mesh AllReduce on a 1KB buffer.
            # Descriptor generation (7 x ~830ns of Pool work) happens HERE,
            # at kernel start, on SWDGE queue 1; only the trigger waits for
            # the stats. The entry-barrier / arrival waits are attached
            # post-scheduling (below) — the tile scheduler's single-core sim
            # cannot satisfy remotely-incremented semaphores.
            for k in range(1, N_CORES):
                rd = [None] * 8
                rd[k] = (0, k)
                nc.gpsimd.remote_dma_broadcast(
                    out_ap=recv[:, 2 * k:2 * k + 2], in_ap=stats[:, 0:2],
                    remote_sem=sem_arrive, local_sem=sem_sent, rdests=rd)

            # ---------- sweep 0: inhibitory ----------
            emit_sweep(0, inh3)
            nc.vector.reduce_sum(stats[:, 0:1], inh[:],
                                 axis=mybir.AxisListType.X)
            nc.vector.scalar_tensor_tensor(
                spk[:], inh[:], 0.0, inh[:], ALU.bypass, ALU.mult,
                accum_out=stats[:, 1:2])
            # Order the trigger after the stats write: a Pool-engine read of
            # stats into a dummy tile (RAW on stats), then a WAW between that
            # dummy and the trigger's signals_writable.
            dummy = singles.tile([NI, 2], F32)
            nc.gpsimd.tensor_copy(out=dummy[:], in_=stats[:])
            stats_trigger = nc.gpsimd.trigger_dma(
                count=None, signals_writable=[dummy[:]])
            nc.vector.tensor_copy(out=recv[:, 0:2], in_=stats[:, 0:2])

            # ---------- sweeps 1+2: excitatory ----------
            emit_sweep(1, exc0, dma_drain=True)
            emit_sweep(2, exc1, dma_drain=True)

            # BN math after the stats arrive from all peers (wait rides on
            # the first reduce op). Everything downstream of the remote wait
            # is emitted AFTER both exc sweeps so the scheduler cannot place
            # remote-gated ops mid-stream, where their in-order engines would
            # block tick-based waits of unrelated conv work.
            sg = smalls[:, 4:5]
            b2 = smalls[:, 6:7]
            stats_reduce = nc.vector.tensor_add(
                recv[:, 0:8], recv[:, 0:8], recv[:, 8:16])
            nc.vector.tensor_add(recv[:, 0:4], recv[:, 0:4], recv[:, 4:8])
            nc.vector.tensor_add(recv[:, 0:2], recv[:, 0:2], recv[:, 2:4])
            ninv = 1.0 / (N_LOC * N_CORES)
            nc.vector.tensor_scalar_mul(gst[:], recv[:, 0:2], ninv)
            gmean = gst[:, 0:1]
            gex2 = gst[:, 1:2]
            msq = smalls[:, 0:1]
            nc.vector.tensor_mul(msq, gmean, gmean)
            var = smalls[:, 1:2]
            nc.vector.tensor_sub(var, gex2, msq)
            stdv = smalls[:, 2:3]
            nc.scalar.activation(stdv, var, ACTF.Sqrt, bias=eps_c)
            rstd = smalls[:, 3:4]
            nc.vector.reciprocal(rstd, stdv)
            nc.vector.tensor_mul(sg, rstd, bng[:])
            ms = smalls[:, 5:6]
            nc.vector.tensor_mul(ms, gmean, sg)
            nc.vector.tensor_sub(b2, bnb[:], ms)

            # ---------- BN apply + LIF scan (Vector, overlaps sweep 2) ----
            nc.vector.scalar_tensor_tensor(
                inh[:], inh[:], sg, b2.broadcast_to([NI, N_LOC]),
                ALU.mult, ALU.add)
            for t_i in range(TP):
                vsl = inh3[:, :, t_i]
                nc.vector.scalar_tensor_tensor(
                    vsl, w_st[:], A_DECAY, vsl, ALU.mult, ALU.add)
                nc.vector.scalar_tensor_tensor(
                    w_st[:], vsl, VTH, vsl, ALU.is_lt, ALU.mult)
            nc.vector.tensor_single_scalar(spk[:], inh[:], VTH, ALU.is_ge)

            # ---------- inhibitory linear for both exc halves ----------
            # All lin matmuls sit after every conv in the in-order PE queue,
            # so a wait for spikes cannot stall conv work. Lin tiles reuse
            # the (drained) conv PSUM pool; adds run on Vector (GpSimd
            # cannot read PSUM); stores fire per half (exc0 on the ACT
            # ring, exc1 on the sync ring).
            for half, (dstE, o0) in enumerate(((exc0, 0), (exc1, 128))):
                lw = wei_neg[:, o0:o0 + 128]
                for b_ in range(BL):
                    lp = lpool.tile([128, TP], F32, tag="lin",
                                    name=f"l{half}{b_}")
                    nc.tensor.matmul(lp[:], lw, spk[:, b_ * TP:(b_ + 1) * TP],
                                     start=True, stop=True)
                    nc.vector.tensor_add(dstE[:, b_, :], dstE[:, b_, :],
                                         lp[:])
                    if b_ % 4 == 3:
                        if half == 0:
                            nc.scalar.dma_start(
                                out=out_d.ap()[0:128, b_ - 3:b_ + 1, :],
                                in_=exc0[:, b_ - 3:b_ + 1, :])
                        else:
                            nc.sync.dma_start(
                                out=out_d.ap()[128:256, b_ - 3:b_ + 1, :],
                                in_=exc1[:, b_ - 3:b_ + 1, :])

    # Post-scheduling: attach the HW-only semaphore waits the scheduler's
    # sim can't satisfy. The trigger waits for the kernel-entry barrier
    # (all peers started this run, so their preamble state is clean); the
    # first reduce op waits for all 7 peers' stats to have landed.
    stats_trigger.wait_op(nc._bir_kernel_barrier_sem,
                          nc.bir_kernel_barrier_sem_inc, "sem-ge", check=False)
    nc._bir_kernel_barrier_sem_replica_groups.append(set(range(N_CORES)))
    stats_reduce.wait_op(sem_arrive, 2 * (N_CORES - 1), "sem-ge", check=False)
    # After the tile epilogue's all-engine barrier: reset the exchange
    # semaphores so repeated executions of this NEFF start from zero.
    nc.clear_and_free_semaphores([sem_arrive, sem_sent])
    nc.compile()
    return nc


def kernel(x, W_inh, P_inh, SIG_inh, W_exc, P_exc, SIG_exc, w_exc_inh,
           bn_gamma, bn_beta):
    ke = _build_dcls_host(np.asarray(W_exc), np.asarray(P_exc),
                          np.asarray(SIG_exc))        # (256, 700, D)
    ki = _build_dcls_host(np.asarray(W_inh), np.asarray(P_inh),
                          np.asarray(SIG_inh))        # (128, 700, D)
    wins_e = _chunk_windows(ke, BUDGET_EXC)
    wins_i = _chunk_windows(ki, BUDGET_INH)
    kall = np.concatenate([ke, ki], axis=0)
    # sweeps: (o_offset into kall, per-chunk windows) in order inh, exc0, exc1
    sched = ((256, wins_i), (0, wins_e), (128, wins_e))

    if _CACHE.get("key") != sched:
        _CACHE["nc"] = _build_nc(sched)
        _CACHE["key"] = sched
    nc = _CACHE["nc"]

    kt = _pack_segments(kall, sched).astype(np.float16)
    x = np.ascontiguousarray(
        np.asarray(x, dtype=np.float32).astype(np.float16))
    wei = np.ascontiguousarray(
        -np.abs(np.asarray(w_exc_inh, dtype=np.float32)).T
        .astype(np.float16))
    bng = np.asarray(bn_gamma, dtype=np.float32).reshape(NI, 1)
    bnb = np.asarray(bn_beta, dtype=np.float32).reshape(NI, 1)

    shared = {"kt": kt, "wei": wei, "bng": bng, "bnb": bnb}
    in_maps = []
    for c in range(N_CORES):
        m = dict(shared)
        m["xs"] = np.ascontiguousarray(x[c * BL:(c + 1) * BL])
        in_maps.append(m)

    _CACHE["in_maps"] = in_maps
    res = bass_utils.run_bass_kernel_spmd(nc, in_maps,
                                          core_ids=list(range(N_CORES)))
    # device emits (NE, BL, TP); transpose back to (BL, NE, TP) per core
    out = np.concatenate(
        [np.transpose(res.results[c]["out"], (1, 0, 2))
         for c in range(N_CORES)], axis=0)
    return np.ascontiguousarray(out, dtype=np.float32)



# revision 48
# speedup vs baseline: 1.1225x; 1.0081x over previous
# Trainium2 Bass kernel for nn_DCLS_semi_DANNLayer (DCLS gaussian convs + BN +
# LIF scan + inhibitory linear), data-parallel over batch on 8 NeuronCores.
#
# Design notes:
# - DCLS kernels are built exactly on the host and DMA'd in; taps whose
#   gaussian weight is negligible are skipped with a per-(branch, chunk)
#   error-budgeted window computed from the actual inputs at compile time.
# - x is loaded contiguously (5 channels per partition; kernel rows permuted
#   on host to match) so DMA descriptors are 6KB, not 1.2KB.
# - The leftover 60-channel chunk packs two taps per matmul via a one-tap-
#   shifted x copy in partitions 60:120 (filled by an SBUF->SBUF DMA).
# - Matmuls cover two batches x 256 t (512-col PSUM bank); the 20-col tails
#   run once per unit over all 8 batches. The inhibitory linear for the
#   second exc slice accumulates (negated weights) directly into conv PSUM.
# - PSUM drains run on the Scalar engine; BN + the 276-step LIF scan run on
#   Vector, shadowed under the exc sweeps; BN stats are all-reduced across
#   cores while the first exc sweep runs.
#
# Self-contained: hardcodes all shapes; takes FULL inputs, returns FULL output.
import numpy as np

import concourse.bacc as bacc
import concourse.bass as bass
import concourse.mybir as mybir
import concourse.tile as tile
from concourse import bass_utils


# ---- problem constants (hardcoded per spec) ----
N_CORES = 8
B, CI, T = 64, 700, 300
D = 25
TP = T - D + 1            # 276
NE, NI = 256, 128
BL = B // N_CORES         # 8 batches per core
N_LOC = BL * TP           # 2208, (b, t) layout
TAU = 2.0
A_DECAY = 1.0 - 1.0 / TAU  # 0.5
VTH = 1.0
BN_EPS = 1e-5
LIM = D // 2              # 12
TS = 256                  # per-batch columns in the paired matmul
TR = TP - TS              # 20 tail columns

N_CHUNK = 6               # ch0..ch4 (5-packed channels 0:640) + tail (640:700)
ROWS = [128, 128, 128, 128, 128, 120]
CH_ORDER = (5, 0, 1, 2, 3, 4)

BUDGET_EXC = 0.08         # abs std of dropped-tap noise (output absmax ~100)
BUDGET_INH = 0.002        # fp16 quantization adds ~0.0067 on top of tap drop

F32 = mybir.dt.float32
F32R = mybir.dt.float32r
F16 = mybir.dt.float16
ALU = mybir.AluOpType
ACTF = mybir.ActivationFunctionType

_CACHE: dict = {}


# ---------------------------------------------------------------- host side
def _build_dcls_host(W, P, SIG):
    """Exact DCLS 'gauss' kernel, matching the reference math. (O,I,1)->(O,I,D)"""
    j = np.arange(D, dtype=np.float32)
    Pc = np.clip(P[:, :, 0], -LIM, LIM).astype(np.float32) + np.float32(LIM)
    sig = np.abs(SIG[:, :, 0]).astype(np.float32) + np.float32(0.27)
    g = np.exp(np.float32(-0.5) * ((j[None, None, :] - Pc[..., None]) / sig[..., None]) ** 2)
    g = g / (g.sum(-1, keepdims=True) + np.float32(1e-7))
    return np.abs(W[:, :, 0]).astype(np.float32)[..., None] * g


def _chunk_idx():
    perm = 5 * np.arange(128)
    return [perm + ch for ch in range(5)] + [np.arange(640, 700)]


def _chunk_windows(k, budget):
    """Per-chunk contiguous tap windows (d0, L) such that for every output
    channel the total dropped-tap noise std (x ~ N(0,1)) is within budget."""
    var = np.stack([(k[:, ix, :].astype(np.float64) ** 2).sum(1)
                    for ix in _chunk_idx()], 1)          # (O, 6, D)
    tot = var.sum(1)                                     # (O, D)
    total = tot.sum(1)                                   # (O,)
    B2 = budget * budget
    # minimal global window first
    g0, gL = 0, D
    for L in range(2, D + 1):
        done = False
        for d0 in range(0, D - L + 1):
            if (total - tot[:, d0:d0 + L].sum(1)).max() <= B2:
                g0, gL = d0, L
                done = True
                break
        if done:
            break
    win = [[g0, g0 + gL - 1] for _ in range(N_CHUNK)]
    V = total - tot[:, g0:g0 + gL].sum(1)
    # greedy per-chunk edge shrink
    while True:
        best = None
        for c in range(N_CHUNK):
            a, b_ = win[c]
            if b_ - a + 1 <= 1:
                continue
            for side, d in ((0, a), (1, b_)):
                vmax = (V + var[:, c, d]).max()
                if vmax <= B2 and (best is None or vmax < best[0]):
                    best = (vmax, c, side, d)
        if best is None:
            break
        _, c, side, d = best
        V = V + var[:, c, d]
        if side == 0:
            win[c][0] += 1
        else:
            win[c][1] -= 1
    return tuple((a, b_ - a + 1) for a, b_ in win)


def _sweep_layout(wins):
    """Column offsets per chunk within a sweep's kt block: [tail|ch0..ch4]."""
    np5 = (wins[5][1] + 1) // 2
    offs = [0] * N_CHUNK
    offs[5] = 0
    off = np5 * 128
    for c in range(5):
        offs[c] = off
        off += wins[c][1] * 128
    return offs, off


def _pack_segments(kall, sched):
    idxs = _chunk_idx()
    blocks = []
    for (o0, wins) in sched:
        offs, width = _sweep_layout(wins)
        blk = np.zeros((128, width), dtype=np.float32)
        d5, L5 = wins[5]
        taps5 = list(range(d5, d5 + L5))
        np5 = (L5 + 1) // 2
        buf = np.zeros((128, np5, 128), dtype=np.float32)
        buf[0:60] = np.transpose(kall[o0:o0 + 128][:, idxs[5], :]
                                 [:, :, taps5[0::2]], (1, 2, 0))
        if taps5[1::2]:
            buf[60:120, :len(taps5[1::2])] = np.transpose(
                kall[o0:o0 + 128][:, idxs[5], :][:, :, taps5[1::2]], (1, 2, 0))
        blk[:, 0:np5 * 128] = buf.reshape(128, np5 * 128)
        for c in range(5):
            d0, L = wins[c]
            sub = np.transpose(kall[o0:o0 + 128][:, idxs[c], d0:d0 + L],
                               (1, 2, 0))
            blk[:, offs[c]:offs[c] + L * 128] = sub.reshape(128, L * 128)
        blocks.append(blk)
    return np.concatenate(blocks, axis=1)


# ---------------------------------------------------------------- device side
def _build_nc(sched):
    nc = bacc.Bacc("TRN2", target_bir_lowering=False, debug=False,
                   num_devices=N_CORES)
    # stats exchange semaphores (allocated outside the tile context so the
    # post-context clear can reference them)
    sem_arrive = nc.alloc_semaphore("stats_arrive")
    sem_sent = nc.alloc_semaphore("stats_sent")

    layouts = [_sweep_layout(wins) for (_, wins) in sched]
    widths = [w for (_, w) in layouts]
    bases = [sum(widths[:s]) for s in range(3)]

    xs_d = nc.dram_tensor("xs", [BL, CI, T], F16, kind="ExternalInput")
    kt_d = nc.dram_tensor("kt", [128, sum(widths)], F16, kind="ExternalInput")
    wei_d = nc.dram_tensor("wei", [NI, NE], F16, kind="ExternalInput")
    bng_d = nc.dram_tensor("bng", [NI, 1], F32, kind="ExternalInput")
    bnb_d = nc.dram_tensor("bnb", [NI, 1], F32, kind="ExternalInput")
    # o-major output layout: per-partition rows are BL*TP*4B contiguous, so
    # stores need only 128 large descriptors; host transposes back to (B,O,T)
    out_d = nc.dram_tensor("out", [NE, BL, TP], F32, kind="ExternalOutput")

    def taps_of(s, c):
        d0, L = sched[s][1][c]
        return list(range(d0, d0 + L))

    with tile.TileContext(nc) as tc:
        import contextlib

        with contextlib.ExitStack() as ctx:
            singles = ctx.enter_context(tc.tile_pool(name="singles", bufs=1))
            ktpool = ctx.enter_context(tc.tile_pool(name="ktpool", bufs=10))
            ppool = ctx.enter_context(
                tc.tile_pool(name="ppool", bufs=4, space="PSUM"))
            tpool = ctx.enter_context(
                tc.tile_pool(name="tpool", bufs=2, space="PSUM"))
            lpool = ctx.enter_context(
                tc.tile_pool(name="lpool", bufs=2, space="PSUM"))

            # ---- persistent SBUF tiles ----
            xtm = singles.tile([128, BL, 5, T], F16)   # channels 0:640, 5/part
            xt5 = singles.tile([128, BL, T], F16)      # channels 640:700 + shift
            inh = singles.tile([NI, N_LOC], F32)        # (b, t) layout
            inh3 = inh.rearrange("p (b t) -> p b t", t=TP)
            spk = singles.tile([NI, N_LOC], F16)
            exc0 = singles.tile([128, BL, TP], F32)
            exc1 = singles.tile([128, BL, TP], F32)
            wei_neg = singles.tile([NI, NE], F16)      # -|w_exc_inh|.T (host)
            bng = singles.tile([NI, 1], F32)
            bnb = singles.tile([NI, 1], F32)
            stats = singles.tile([NI, 2], F32)
            gst = singles.tile([NI, 2], F32)
            smalls = singles.tile([NI, 8], F32)
            w_st = singles.tile([NI, BL], F32)
            # per-core stats land here: slot k (cols 2k:2k+2) holds the stats
            # of peer (self XOR k); slot 0 is written locally
            recv = singles.tile([NI, 2 * N_CORES], F32)

            # per-(sweep, chunk) kt tiles: tile-granular DMA dependencies so a
            # matmul only waits for its own chunk's columns
            kt_tiles = {}

            ktw = max((((L_ + 1) // 2) if c_ == 5 else L_) * 128
                      for (_, ws) in sched for c_, (_, L_) in enumerate(ws))

            def load_seg(s, c):
                L = sched[s][1][c][1]
                ncols = (((L + 1) // 2) if c == 5 else L) * 128
                a = layouts[s][0][c]
                t_ = ktpool.tile([128, ktw], F16, tag="kt", name=f"kt{s}{c}")
                nc.sync.dma_start(
                    out=t_[:, :ncols],
                    in_=kt_d.ap()[:, bases[s] + a:bases[s] + a + ncols])
                kt_tiles[(s, c)] = t_

            # ---- head DMAs (sync engine; order = priority) ----
            # sweep-0 pass 0 needs: xt5 batch 0-3, the chunk-5 kernel
            # columns, xtm batch 0, then the remaining sweep-0 chunks.
            def load_x(b_):
                nc.sync.dma_start(out=xtm[:, b_], in_=xs_d.ap()[b_, 0:640]
                                  .rearrange("(p c) t -> p c t", c=5))

            nc.sync.dma_start(
                out=xt5[0:60, 0:4], in_=xs_d.ap()[0:4, 640:700]
                .rearrange("b i t -> i b t"))
            nc.sync.dma_start(out=xt5[60:120, 0:4, 0:T - 1],
                              in_=xt5[0:60, 0:4, 1:T])
            load_seg(0, 5)
            load_x(0)
            for c_ in (0, 1, 2, 3, 4):
                load_seg(0, c_)
            load_x(1)
            load_x(2)
            load_x(3)
            nc.sync.dma_start(
                out=xt5[0:60, 4:8], in_=xs_d.ap()[4:8, 640:700]
                .rearrange("b i t -> i b t"))
            nc.sync.dma_start(out=xt5[60:120, 4:8, 0:T - 1],
                              in_=xt5[0:60, 4:8, 1:T])
            for b_ in range(4, BL):
                load_x(b_)
            nc.sync.dma_start(out=wei_neg[:], in_=wei_d.ap())
            nc.sync.dma_start(out=bng[:], in_=bng_d.ap())
            nc.sync.dma_start(out=bnb[:], in_=bnb_d.ap())
            for s_ in (1, 2):
                for c_ in CH_ORDER:
                    load_seg(s_, c_)

            nc.vector.memset(w_st[:], 0.0)
            eps_c = smalls[:, 7:8]
            nc.vector.memset(eps_c, BN_EPS)

            def rhs(c, b0, nb, t0, t1):
                if c < 5:
                    return xtm[:, b0:b0 + nb, c, t0:t1]
                return xt5[:120, b0:b0 + nb, t0:t1]

            # ---- sweep emitter ----
            # Two-batch 256-col pair matmuls (4 PSUM tiles, tile-major so
            # consecutive matmuls share a bank) plus one 8-batch 20-col tail
            # matmul per unit: 5 instructions per unit instead of 8, which
            # keeps the engines' instruction streams inside the prefetched
            # code pages (the 8-pass form stalled ~10us on page fetches).
            def units_of(s):
                out = []
                for c in CH_ORDER:
                    taps = taps_of(s, c)
                    n_units = len(taps) if c < 5 else (len(taps) + 1) // 2
                    for j in range(n_units):
                        d = taps[j] if c < 5 else taps[2 * j]
                        out.append((c, j, d))
                return out

            def emit_sweep(s, dst3, dma_drain=False):
                units = units_of(s)
                nu = len(units)
                for q in range(2):
                    bA = 4 * q
                    pA = ppool.tile([128, 2, TS], F32, tag="pp",
                                    name=f"pA{s}{q}")
                    pB = ppool.tile([128, 2, TS], F32, tag="pp",
                                    name=f"pB{s}{q}")
                    for pt, b0 in ((pA, bA), (pB, bA + 2)):
                        for i, (c, j, d) in enumerate(units):
                            lhsT = kt_tiles[(s, c)][:ROWS[c],
                                                    j * 128:(j + 1) * 128]
                            nc.tensor.matmul(
                                pt[:], lhsT, rhs(c, b0, 2, d, d + TS),
                                start=(i == 0), stop=(i == nu - 1))
                        nc.scalar.copy(out=dst3[:, b0:b0 + 2, 0:TS],
                                       in_=pt[:])
                tt = tpool.tile([128, BL, TR], F32, tag="tp", name=f"tt{s}")
                for i, (c, j, d) in enumerate(units):
                    lhsT = kt_tiles[(s, c)][:ROWS[c], j * 128:(j + 1) * 128]
                    nc.tensor.matmul(
                        tt[:], lhsT, rhs(c, 0, BL, d + TS, d + TP),
                        start=(i == 0), stop=(i == nu - 1))
                nc.scalar.copy(out=dst3[:, :, TS:TP], in_=tt[:])

            # stats all-gather via direct remote DMA (XOR-relative slots):
            # core c sends its [NI,2] stats into slot k of peer c^k. ~3us
            # vs ~40us for the firmware mesh AllReduce on a 1KB buffer.
            # Descriptor generation (7 x ~830ns of Pool work) happens HERE,
            # at kernel start, on SWDGE queue 1; only the trigger waits for
            # the stats. The entry-barrier / arrival waits are attached
            # post-scheduling (below) — the tile scheduler's single-core sim
            # cannot satisfy remotely-incremented semaphores.
            for k in range(1, N_CORES):
                rd = [None] * 8
                rd[k] = (0, k)
                nc.gpsimd.remote_dma_broadcast(
                    out_ap=recv[:, 2 * k:2 * k + 2], in_ap=stats[:, 0:2],
                    remote_sem=sem_arrive, local_sem=sem_sent, rdests=rd)

            # ---------- sweep 0: inhibitory ----------
            emit_sweep(0, inh3)
            nc.vector.reduce_sum(stats[:, 0:1], inh[:],
                                 axis=mybir.AxisListType.X)
            nc.vector.scalar_tensor_tensor(
                spk[:], inh[:], 0.0, inh[:], ALU.bypass, ALU.mult,
                accum_out=stats[:, 1:2])
            # Order the trigger after the stats write: a Pool-engine read of
            # stats into a dummy tile (RAW on stats), then a WAW between that
            # dummy and the trigger's signals_writable.
            dummy = singles.tile([NI, 2], F32)
            nc.gpsimd.tensor_copy(out=dummy[:], in_=stats[:])
            stats_trigger = nc.gpsimd.trigger_dma(
                count=None, signals_writable=[dummy[:]])
            nc.vector.tensor_copy(out=recv[:, 0:2], in_=stats[:, 0:2])

            # ---------- sweeps 1+2: excitatory ----------
            emit_sweep(1, exc0, dma_drain=True)
            emit_sweep(2, exc1, dma_drain=True)

            # BN math after the stats arrive from all peers (wait rides on
            # the first reduce op). Everything downstream of the remote wait
            # is emitted AFTER both exc sweeps so the scheduler cannot place
            # remote-gated ops mid-stream, where their in-order engines would
            # block tick-based waits of unrelated conv work.
            sg = smalls[:, 4:5]
            b2 = smalls[:, 6:7]
            stats_reduce = nc.vector.tensor_add(
                recv[:, 0:8], recv[:, 0:8], recv[:, 8:16])
            nc.vector.tensor_add(recv[:, 0:4], recv[:, 0:4], recv[:, 4:8])
            nc.vector.tensor_add(recv[:, 0:2], recv[:, 0:2], recv[:, 2:4])
            ninv = 1.0 / (N_LOC * N_CORES)
            nc.vector.tensor_scalar_mul(gst[:], recv[:, 0:2], ninv)
            gmean = gst[:, 0:1]
            gex2 = gst[:, 1:2]
            msq = smalls[:, 0:1]
            nc.vector.tensor_mul(msq, gmean, gmean)
            var = smalls[:, 1:2]
            nc.vector.tensor_sub(var, gex2, msq)
            stdv = smalls[:, 2:3]
            nc.scalar.activation(stdv, var, ACTF.Sqrt, bias=eps_c)
            rstd = smalls[:, 3:4]
            nc.vector.reciprocal(rstd, stdv)
            nc.vector.tensor_mul(sg, rstd, bng[:])
            ms = smalls[:, 5:6]
            nc.vector.tensor_mul(ms, gmean, sg)
            nc.vector.tensor_sub(b2, bnb[:], ms)

            # ---------- BN apply + LIF scan (Vector, overlaps sweep 2) ----
            nc.vector.scalar_tensor_tensor(
                inh[:], inh[:], sg, b2.broadcast_to([NI, N_LOC]),
                ALU.mult, ALU.add)
            for t_i in range(TP):
                vsl = inh3[:, :, t_i]
                nc.vector.scalar_tensor_tensor(
                    vsl, w_st[:], A_DECAY, vsl, ALU.mult, ALU.add)
                nc.vector.scalar_tensor_tensor(
                    w_st[:], vsl, VTH, vsl, ALU.is_lt, ALU.mult)
            for b_ in range(BL):
                nc.vector.tensor_single_scalar(
                    spk[:, b_ * TP:(b_ + 1) * TP],
                    inh[:, b_ * TP:(b_ + 1) * TP], VTH, ALU.is_ge)

            # ---------- inhibitory linear for both exc halves ----------
            # All lin matmuls sit after every conv in the in-order PE queue,
            # so a wait for spikes cannot stall conv work. Lin tiles reuse
            # the (drained) conv PSUM pool; adds run on Vector (GpSimd
            # cannot read PSUM); stores fire per half (exc0 on the ACT
            # ring, exc1 on the sync ring).
            for half, (dstE, o0) in enumerate(((exc0, 0), (exc1, 128))):
                lw = wei_neg[:, o0:o0 + 128]
                for b_ in range(BL):
                    lp = lpool.tile([128, TP], F32, tag="lin",
                                    name=f"l{half}{b_}")
                    nc.tensor.matmul(lp[:], lw, spk[:, b_ * TP:(b_ + 1) * TP],
                                     start=True, stop=True)
                    nc.vector.tensor_add(dstE[:, b_, :], dstE[:, b_, :],
                                         lp[:])
                    if b_ % 4 == 3:
                        if half == 0:
                            nc.scalar.dma_start(
                                out=out_d.ap()[0:128, b_ - 3:b_ + 1, :],
                                in_=exc0[:, b_ - 3:b_ + 1, :])
                        else:
                            nc.sync.dma_start(
                                out=out_d.ap()[128:256, b_ - 3:b_ + 1, :],
                                in_=exc1[:, b_ - 3:b_ + 1, :])

    # Post-scheduling: attach the HW-only semaphore waits the scheduler's
    # sim can't satisfy. The trigger waits for the kernel-entry barrier
    # (all peers started this run, so their preamble state is clean); the
    # first reduce op waits for all 7 peers' stats to have landed.
    stats_trigger.wait_op(nc._bir_kernel_barrier_sem,
                          nc.bir_kernel_barrier_sem_inc, "sem-ge", check=False)
    nc._bir_kernel_barrier_sem_replica_groups.append(set(range(N_CORES)))
    stats_reduce.wait_op(sem_arrive, 2 * (N_CORES - 1), "sem-ge", check=False)
    # After the tile epilogue's all-engine barrier: reset the exchange
    # semaphores so repeated executions of this NEFF start from zero.
    nc.clear_and_free_semaphores([sem_arrive, sem_sent])
    nc.compile()
    return nc


def kernel(x, W_inh, P_inh, SIG_inh, W_exc, P_exc, SIG_exc, w_exc_inh,
           bn_gamma, bn_beta):
    ke = _build_dcls_host(np.asarray(W_exc), np.asarray(P_exc),
                          np.asarray(SIG_exc))        # (256, 700, D)
    ki = _build_dcls_host(np.asarray(W_inh), np.asarray(P_inh),
                          np.asarray(SIG_inh))        # (128, 700, D)
    wins_e = _chunk_windows(ke, BUDGET_EXC)
    wins_i = _chunk_windows(ki, BUDGET_INH)
    kall = np.concatenate([ke, ki], axis=0)
    # sweeps: (o_offset into kall, per-chunk windows) in order inh, exc0, exc1
    sched = ((256, wins_i), (0, wins_e), (128, wins_e))

    if _CACHE.get("key") != sched:
        _CACHE["nc"] = _build_nc(sched)
        _CACHE["key"] = sched
    nc = _CACHE["nc"]

    kt = _pack_segments(kall, sched).astype(np.float16)
    x = np.ascontiguousarray(
        np.asarray(x, dtype=np.float32).astype(np.float16))
    wei = np.ascontiguousarray(
        -np.abs(np.asarray(w_exc_inh, dtype=np.float32)).T
        .astype(np.float16))
    bng = np.asarray(bn_gamma, dtype=np.float32).reshape(NI, 1)
    bnb = np.asarray(bn_beta, dtype=np.float32).reshape(NI, 1)

    shared = {"kt": kt, "wei": wei, "bng": bng, "bnb": bnb}
    in_maps = []
    for c in range(N_CORES):
        m = dict(shared)
        m["xs"] = np.ascontiguousarray(x[c * BL:(c + 1) * BL])
        in_maps.append(m)

    _CACHE["in_maps"] = in_maps
    res = bass_utils.run_bass_kernel_spmd(nc, in_maps,
                                          core_ids=list(range(N_CORES)))
    # device emits (NE, BL, TP); transpose back to (BL, NE, TP) per core
    out = np.concatenate(
        [np.transpose(res.results[c]["out"], (1, 0, 2))
         for c in range(N_CORES)], axis=0)
    return np.ascontiguousarray(out, dtype=np.float32)

